# revision 6
# baseline (speedup 1.0000x reference)
"""MoE gate (group-limited top-k routing) as a Bass/Tile kernel for 8 TRN2 cores.

Computes, per token:
  logits = hidden @ W            (K=7168, E=256)
  scores = sigmoid(logits) + bias
  group-limited routing: top-2-sum per group of 32 -> top-4 groups of 8
  top-8 of masked scores, renormalized, * 2.5

Sharding: data-parallel over tokens (1024 tokens/core), W + bias replicated.

The device kernel takes hidden and W already cast to fp16 (the cast happens
host-side in `kernel()` as part of staging the shards) which halves HBM
traffic.  The fp16 mantissa (11 bits) keeps the logit error ~1e-3 absolute
against a ~1.7 logit std; PSUM accumulation is fp32.

Matmul layout: hidden tiles are loaded TRANSPOSED straight from DRAM via the
DMA XBAR (dma_start_transpose, 2-byte dtype), so the tensor engine runs only
the 448 gating matmuls (56 K-chunks x 8 token tiles, fp16 = 1 cycle/row) and
the vector/scalar engines only run the routing epilogue.  Transpose loads
alternate between the sync and scalar HWDGE queues.
"""

import sys

if "/opt/trn_rl_repo" not in sys.path:
    sys.path.insert(0, "/opt/trn_rl_repo")

import numpy as np

import concourse.bacc as bacc
import concourse.bass as bass
import concourse.mybir as mybir
import concourse.tile as tile
from concourse import bass_utils

P = 128
TOP_K = 8
N_GROUP = 8
TOPK_GROUP = 4
SCALE = 2.5

N_CORES = 8
TOKENS = 8192
HIDDEN = 7168
EXPERTS = 256


def build_moe_gate(
    tokens_per_core=TOKENS // N_CORES,
    hidden=HIDDEN,
    n_experts=EXPERTS,
    scheme="xbar",
):
    KC = hidden // P           # K-chunks of 128 (56)
    TT = tokens_per_core // P  # token tiles of 128 (8)
    GS = n_experts // N_GROUP  # experts per group (32)
    f32 = mybir.dt.float32
    f16 = mybir.dt.float16

    nc = bacc.Bacc("TRN2", target_bir_lowering=False, debug=False)
    hs = nc.dram_tensor(
        "hidden_states", [tokens_per_core, hidden], f16, kind="ExternalInput"
    ).ap()
    wk = nc.dram_tensor("kernel", [hidden, n_experts], f16, kind="ExternalInput").ap()
    bias = nc.dram_tensor(
        "e_score_correction_bias", [n_experts], f32, kind="ExternalInput"
    ).ap()
    out = nc.dram_tensor(
        "topk_out", [tokens_per_core, TOP_K], f32, kind="ExternalOutput"
    ).ap()

    with tile.TileContext(nc) as tc:
        with (
            tc.tile_pool(name="const", bufs=1) as cpool,
            tc.tile_pool(name="hT", bufs=10) as hTpool,
            tc.tile_pool(name="plog", bufs=1, space="PSUM") as plpool,
            tc.tile_pool(name="route", bufs=3) as rpool,
        ):
            # --- resident replicated weights (fp16, direct DMA, no prep) ---
            wsb = cpool.tile([P, KC, n_experts], f16)
            wk_view = wk.rearrange("(kc p) e -> p kc e", p=P)
            HKC = KC // 2
            # k-ordered halves so chunk-0 matmuls can start early
            nc.sync.dma_start(out=wsb[:, :HKC, :], in_=wk_view[:, :HKC, :])
            nc.scalar.dma_start(out=wsb[:, HKC:, :], in_=wk_view[:, HKC:, :])

            bias_sb = cpool.tile([P, n_experts], f32)
            bias_bcast = bass.AP(
                tensor=bias.tensor, offset=bias.offset, ap=[[0, P]] + list(bias.ap)
            )
            nc.gpsimd.dma_start(out=bias_sb, in_=bias_bcast)

            # logits accumulate in PSUM for all TT token tiles across the
            # whole K loop; one bank per tile — a PSUM bank (2KB zero region)
            # supports only ONE pending accumulation group at a time
            lg = [
                plpool.tile([P, n_experts], f32, name=f"lg{i}") for i in range(TT)
            ]

            wout_all = cpool.tile([P, TT, TOP_K], f32)

            for k in range(KC):
                hTk = hTpool.tile([P, tokens_per_core], f16)
                eng = nc.sync if k % 2 == 0 else nc.scalar
                eng.dma_start_transpose(hTk, hs[:, k * P : (k + 1) * P])
                for t in range(TT):
                    nc.tensor.matmul(
                        lg[t],
                        lhsT=hTk[:, t * P : (t + 1) * P],
                        rhs=wsb[:, k, :],
                        start=(k == 0),
                        stop=(k == KC - 1),
                    )

            # ---- routing epilogue (tokens on partitions) ----
            for t in range(TT):
                sc = rpool.tile([P, n_experts], f32)
                nc.scalar.activation(
                    sc, lg[t], mybir.ActivationFunctionType.Sigmoid
                )
                nc.vector.tensor_add(sc, sc, bias_sb)

                # top-2 sum per group of GS experts
                m8 = rpool.tile([P, N_GROUP * 8], f32)
                for g in range(N_GROUP):
                    nc.vector.max(
                        m8[:, g * 8 : (g + 1) * 8], sc[:, g * GS : (g + 1) * GS]
                    )
                m8v = m8.rearrange("p (g k) -> p g k", k=8)
                gsum = rpool.tile([P, N_GROUP], f32)
                nc.vector.tensor_add(gsum, m8v[:, :, 0], m8v[:, :, 1])

                # top-TOPK_GROUP groups -> per-group 0/1 mask via threshold
                gmax = rpool.tile([P, 8], f32)
                nc.vector.max(gmax, gsum)
                gmask = rpool.tile([P, N_GROUP], f32)
                nc.vector.tensor_scalar(
                    gmask,
                    gsum,
                    gmax[:, TOPK_GROUP - 1 : TOPK_GROUP],
                    None,
                    op0=mybir.AluOpType.is_ge,
                )

                # masked scores = sc * mask (0 where group dropped)
                masked = rpool.tile([P, n_experts], f32)
                nc.vector.tensor_mul(
                    masked.rearrange("p (g e) -> p g e", g=N_GROUP),
                    sc.rearrange("p (g e) -> p g e", g=N_GROUP),
                    gmask[:, :, None].broadcast_to([P, N_GROUP, GS]),
                )

                top8 = rpool.tile([P, TOP_K], f32)
                nc.vector.max(top8, masked)

                dsum = rpool.tile([P, 1], f32)
                nc.vector.reduce_sum(dsum, top8, axis=mybir.AxisListType.X)
                rcp = rpool.tile([P, 1], f32)
                nc.vector.reciprocal(rcp, dsum)
                nc.vector.tensor_scalar(
                    wout_all[:, t, :],
                    top8,
                    rcp,
                    SCALE,
                    op0=mybir.AluOpType.mult,
                    op1=mybir.AluOpType.mult,
                )

            nc.sync.dma_start(
                out=out.rearrange("(tt p) k -> p tt k", p=P), in_=wout_all
            )

    nc.compile()
    return nc


_CACHE = {}


def _built_nc():
    if "nc" not in _CACHE:
        _CACHE["nc"] = build_moe_gate()
    return _CACHE["nc"]


def kernel(hidden_states, kernel, e_score_correction_bias):
    hs = np.ascontiguousarray(np.asarray(hidden_states, dtype=np.float32))
    wk = np.ascontiguousarray(np.asarray(kernel, dtype=np.float32))
    bi = np.ascontiguousarray(np.asarray(e_score_correction_bias), dtype=np.float32)
    assert hs.shape == (TOKENS, HIDDEN) and wk.shape == (HIDDEN, EXPERTS)

    # stage the device shards in fp16 (halves HBM traffic; see module doc)
    hs16 = hs.astype(np.float16)
    wk16 = wk.astype(np.float16)

    tpc = TOKENS // N_CORES
    nc = _built_nc()
    in_maps = [
        {
            "hidden_states": hs16[i * tpc : (i + 1) * tpc],
            "kernel": wk16,
            "e_score_correction_bias": bi,
        }
        for i in range(N_CORES)
    ]
    res = bass_utils.run_bass_kernel_spmd(nc, in_maps, core_ids=list(range(N_CORES)))
    return np.concatenate(
        [res.results[i]["topk_out"] for i in range(N_CORES)], axis=0
    )
